# revision 1
# baseline (speedup 1.0000x reference)
"""Distributed causal self-attention kernel for 8 Trainium2 NeuronCores.

Problem: B=4, T=2048, C=1024, H=16 heads, D=64 head dim, fp32.
  qkv = x @ W_qkv.T + b_qkv; causal attention per head; out = attn @ W_proj.T + b_proj

Sharding (hybrid DP x TP, no on-device collectives):
  core c -> batch b = c//2 (data parallel), head group g = c%2 (8 heads each,
  tensor parallel). Each core computes a row-parallel *partial* projection
  output for its batch; the host sums the two partials per batch (the TP
  reduction) and adds b_proj. All weights are pre-transposed / pre-scaled on
  the host so the device only runs matmuls in their natural layouts:

  - xT [C, T]: x[b] transposed -> moving/stationary operand with contraction
    (C) on partitions.
  - Q^T, K^T produced in [j, T] layout (weight-stationary matmuls); the 1/8
    attention scale is folded into W_q/b_q on the host.
  - V produced in natural [T, j] layout (x-stationary matmuls), stored per
    k-tile as [ones(64) | V_0..V_7] so the attn@V stationary [ones|V_h]
    computes the softmax denominator (replicated on partitions 0-63) and the
    un-normalized output (partitions 64-127) in a single matmul.
  - Scores are computed TRANSPOSED (scores_T[t_k, t_q] = K^T.T @ Q^T) so that
    P~ = exp(scores_T) is directly the moving operand of attn@V -- no
    transposes anywhere in the attention pipeline.
  - proj consumes attnT [j, T] directly, producing outT [C, T] partials.

  All matmul operands use float32r (tf32-like, 4x faster than fp32 on the
  PE; rel err ~1e-3 end-to-end).
"""
import sys

if "/opt/trn_rl_repo" not in sys.path:
    sys.path.insert(0, "/opt/trn_rl_repo")

import ml_dtypes
import numpy as np

import concourse.bass as bass
import concourse.tile as tile
from concourse import bacc, mybir
from concourse.bass_utils import run_bass_kernel_spmd
from concourse.masks import make_upper_triangular

F32 = mybir.dt.float32
F32R = mybir.dt.float32r
BF16 = mybir.dt.bfloat16

B, T, C = 4, 2048, 1024
H, D = 16, 64
HC = 8            # heads per core
P = 128           # partitions
NCORES = 8
NT = T // P       # 16 t-tiles of 128
NTC = T // 512    # 4 t-chunks of 512
NCT = C // P      # 8 contraction tiles for qkv
JQK = 1024        # q+k columns per core
NJT = JQK // P    # 8 j-tiles (4 q, 4 k)
JV = 512          # v columns per core
NMT = C // P      # 8 proj output row tiles
NPJ = JV // P     # 4 proj contraction tiles

_compiled = None


def build():
    nc = bacc.Bacc("TRN2", target_bir_lowering=False, debug=False,
                   num_devices=NCORES)
    x_ext = nc.declare_dram_parameter("xT", [C, T], F32R, isOutput=False)
    wqkv_ext = nc.declare_dram_parameter("wqkv", [C, 3 * JV], F32R, isOutput=False)
    bqkv_ext = nc.declare_dram_parameter("bqkv", [3 * JV], F32, isOutput=False)
    wproj_ext = nc.declare_dram_parameter("wproj", [JV, C], BF16, isOutput=False)
    bproj_ext = nc.declare_dram_parameter("bproj", [C], F32, isOutput=False)
    out_ext = nc.declare_dram_parameter("out", [C, T], F32, isOutput=True)

    with tile.TileContext(nc, pool_alloc_mode="queue") as tc:
        _body(nc, tc, x_ext, wqkv_ext, bqkv_ext, wproj_ext, bproj_ext, out_ext)
    nc.compile()
    return nc


def _body(nc, tc, x_ext, wqkv_ext, bqkv_ext, wproj_ext, bproj_ext, out_ext):
    dma = nc.default_dma_engine

    from contextlib import ExitStack
    ctx = ExitStack()
    with ctx:
        singles = ctx.enter_context(tc.tile_pool(name="singles", bufs=1))
        qkt_pool = ctx.enter_context(tc.tile_pool(name="qkT", bufs=1))
        vpool = ctx.enter_context(tc.tile_pool(name="v", bufs=1))
        apool = ctx.enter_context(tc.tile_pool(name="attnT", bufs=1))
        ptpool = ctx.enter_context(tc.tile_pool(name="pt", bufs=4))
        rspool = ctx.enter_context(tc.tile_pool(name="rs", bufs=4))
        wp_pool = ctx.enter_context(tc.tile_pool(name="wp", bufs=1))
        opool = ctx.enter_context(tc.tile_pool(name="outs", bufs=2))
        psum = ctx.enter_context(tc.tile_pool(name="psum", bufs=1, space="PSUM"))
        xpool_cm = tc.tile_pool(name="x", bufs=1, side="right")
        xpool = xpool_cm.__enter__()
        wv_cm = tc.tile_pool(name="wv", bufs=1, side="right")
        wv_pool = wv_cm.__enter__()

        # ---- HAM warmup: dummy fp32 matmuls (4 cyc/row, ~1.7us each when
        # cold) bridge the initial DMA ramp so the PE clock is at 8/8 when
        # the first real matmuls issue. Reuses an rs-pool slot: no extra SBUF.
        warm = rspool.tile([P, 512], F32, tag="rs", name="warm")
        nc.vector.memset(warm[:], 1.0)
        for i in range(6):
            wps = psum.tile([P, 512], F32, tag="mm", bufs=2, name=f"warm{i}")
            nc.tensor.matmul(wps[:], warm[:, 0:P], warm[:])

        # ---- constants ----
        mask = singles.tile([P, P], BF16)       # upper-tri (t_q >= t_k) 0/1
        make_upper_triangular(nc, mask[:], val=1.0, diag=True)

        bqk_t = singles.tile([P, NJT], F32)     # per-partition q/k biases
        dma.dma_start(out=bqk_t[:], in_=bqkv_ext[:JQK].rearrange("(j p) -> p j", p=P))
        bv_b = singles.tile([P, JV], F32)       # v bias broadcast over partitions
        bv_src = bass.AP(tensor=bqkv_ext, offset=JQK, ap=[[0, P], [1, JV]])
        dma.dma_start(out=bv_b[:], in_=bv_src)
        bproj_t = singles.tile([P, NMT], F32)
        dma.dma_start(out=bproj_t[:], in_=bproj_ext[:].rearrange("(m p) -> p m", p=P))

        # ---- x loads (column-chunked so the v pass can start early) ----
        wv = []
        for ct in range(NCT):
            wt = wv_pool.tile([P, JV], F32R, tag=f"wv{ct}", name=f"wv{ct}")
            dma.dma_start(out=wt[:, 0:256],
                          in_=wqkv_ext[ct * P:(ct + 1) * P, JQK:JQK + 256])
            dma.dma_start(out=wt[:, 256:],
                          in_=wqkv_ext[ct * P:(ct + 1) * P, JQK + 256:])
            wv.append(wt)
        xts = [xpool.tile([P, T], F32R, tag=f"x{ct}", name=f"x{ct}")
               for ct in range(NCT)]
        for tcn in range(NTC):
            for ct in range(NCT):
                dma.dma_start(
                    out=xts[ct][:, tcn * 512:(tcn + 1) * 512],
                    in_=x_ext[ct * P:(ct + 1) * P, tcn * 512:(tcn + 1) * 512])

        # ---- v pass: 16 k-tiles ----
        # v_sb[kt]: [128, 1024] = per head h: [ones(64) | V_h(64)] at col 128h;
        # ones make attn@V emit the softmax denominator on partitions 0-63
        vts = []

        def v_tile(kt):
            psv = psum.tile([P, JV], F32, tag="mm", bufs=2, name=f"psv{kt}")
            for ct in range(NCT):
                nc.tensor.matmul(
                    psv[:], xts[ct][:, kt * P:(kt + 1) * P], wv[ct][:],
                    start=(ct == 0), stop=(ct == NCT - 1),
                )
            vt = vpool.tile([P, 2 * JV], BF16, tag=f"v{kt}", name=f"v{kt}")
            vt3 = vt[:].rearrange("p (h c) -> p h c", h=HC)
            nc.vector.memset(vt3[:, :, 0:64], 1.0)
            nc.vector.tensor_add(
                vt3[:, :, 64:128],
                psv[:].rearrange("p (h c) -> p h c", h=HC),
                bv_b[:].rearrange("p (h c) -> p h c", h=HC),
            )
            vts.append(vt)

        for kt in range(4):
            v_tile(kt)

        with tc.tile_pool(name="wqk", bufs=1, side="right") as wqk_pool:
            wqk = []
            for ct in range(NCT):
                wt = wqk_pool.tile([P, JQK], F32R, tag=f"wqk{ct}", name=f"wqk{ct}")
                dma.dma_start(out=wt[:], in_=wqkv_ext[ct * P:(ct + 1) * P, :JQK])
                wqk.append(wt)
            wproj = []
            for jt in range(NPJ):
                wt = wp_pool.tile([P, C], BF16, tag=f"wp{jt}", name=f"wp{jt}")
                dma.dma_start(out=wt[:], in_=wproj_ext[jt * P:(jt + 1) * P, :])
                wproj.append(wt)

            # q/k tiles rotate between even/odd pairs (2 pairs in flight)
            qkT = {}
            attnT = [apool.tile([P, T], BF16, tag=f"a{p_}", name=f"attnT{p_}")
                     for p_ in range(4)]

            def qk_tiles(p_):
                qkT[p_] = qkt_pool.tile([P, T], BF16, tag=f"q{p_ % 2}",
                                        name=f"qT{p_}")
                qkT[4 + p_] = qkt_pool.tile([P, T], BF16, tag=f"k{p_ % 2}",
                                            name=f"kT{p_}")
                for jt in (p_, 4 + p_):
                    for tcn in range(NTC):
                        ps = psum.tile([P, 512], F32, tag="mm", bufs=2,
                                       name=f"psqk{jt}_{tcn}")
                        for ct in range(NCT):
                            nc.tensor.matmul(
                                ps[:], wqk[ct][:, jt * P:(jt + 1) * P],
                                xts[ct][:, tcn * 512:(tcn + 1) * 512],
                                start=(ct == 0), stop=(ct == NCT - 1),
                            )
                        nc.vector.tensor_scalar_add(
                            out=qkT[jt][:, tcn * 512:(tcn + 1) * 512],
                            in0=ps[:], scalar1=bqk_t[:, jt:jt + 1])

            # ---- per pair: its two qk j-tiles, then its attention ----
            # The ACT exp chain of pair p overlaps the PE qk matmuls of
            # pair p+1 (lower priority, dependency-free). Pair 0's qk is
            # emitted after only 4 v tiles so its attention (and the ACT
            # pipeline) starts early; remaining v tiles fill PE gaps.
            for p_ in range(4):
                qk_tiles(p_)
                if p_ == 0:
                    for kt in range(4, NT):
                        v_tile(kt)

                qTt = qkT[p_]
                kTt = qkT[4 + p_]
                # pair 3 runs q-chunks high-to-low so the projection (which
                # consumes chunks in the same order) tails on the SHORT
                # qc=0 block instead of the 16-ktile qc=3 block
                qcs = range(NTC) if p_ < 3 else range(NTC - 1, -1, -1)
                for qc in qcs:
                    pso = [psum.tile([P, 512], F32, tag="o", bufs=2,
                                     name=f"pso{p_}_{qc}_{i}")
                           for i in range(2)]
                    nkt = 4 * (qc + 1)
                    pending = None
                    for kt in range(nkt):
                        o = max(0, kt * P - qc * 512)
                        ss = psum.tile([P, 1024], F32, tag="s", bufs=2,
                                       name=f"pss{p_}_{qc}_{kt}")
                        for h in range(2):
                            lo, hi = h * 64, (h + 1) * 64
                            nc.tensor.matmul(
                                ss[:, 512 * h + o:512 * (h + 1)],
                                kTt[lo:hi, kt * P:(kt + 1) * P],
                                qTt[lo:hi, qc * 512 + o:(qc + 1) * 512],
                            )
                        pt = ptpool.tile([P, 1024], BF16, tag="pt",
                                         name=f"pt{p_}_{qc}_{kt}")
                        ss3 = ss[:].rearrange("p (h w) -> p h w", h=2)
                        pt3 = pt[:].rearrange("p (h w) -> p h w", h=2)
                        nc.scalar.activation(
                            pt3[:, :, o:], ss3[:, :, o:],
                            mybir.ActivationFunctionType.Exp,
                        )
                        if kt >= 4 * qc:
                            # diagonal block: zero t_q < t_k (both heads)
                            mask_b = bass.AP(
                                tensor=mask[:].tensor, offset=mask[:].offset,
                                ap=[mask[:].ap[0], [0, 2], [1, P]])
                            nc.vector.tensor_mul(
                                pt3[:, :, o:o + P], pt3[:, :, o:o + P], mask_b)
                        if pending is not None:
                            _emit_av(nc, vts, pso, p_, *pending, nkt)
                        pending = (pt, o, kt)
                    _emit_av(nc, vts, pso, p_, *pending, nkt)
                    # normalize: pso rows 0:64 = row-sum, 64:128 = outT
                    for h in range(2):
                        rsb = rspool.tile([P, 512], F32, tag="rs",
                                          name=f"rs{p_}_{qc}_{h}")
                        # fast recip is lane-locked: compute at base 0 (frees
                        # the psum fast), DMA-shift to partitions 64-127
                        nc.vector.reciprocal_approx_fast(
                            rsb[0:64, :], pso[h][0:64, :])
                        dma.dma_start(out=rsb[64:128, :], in_=rsb[0:64, :])
                        lo = 64 * h
                        nc.vector.tensor_mul(
                            attnT[p_][lo:lo + 64, qc * 512:(qc + 1) * 512],
                            pso[h][64:128, :], rsb[64:128, :])

            # ---- projection: outT[m, t] partial; t-chunk outer, matching
            # pair 3's reversed qc order so each chunk unlocks asap ----
            for tcn in range(NTC - 1, -1, -1):
                for mt in range(NMT):
                    psp = psum.tile([P, 512], F32, tag="mm", bufs=2,
                                    name=f"psp{mt}_{tcn}")
                    for jt in range(NPJ):
                        nc.tensor.matmul(
                            psp[:], wproj[jt][:, mt * P:(mt + 1) * P],
                            attnT[jt][:, tcn * 512:(tcn + 1) * 512],
                            start=(jt == 0), stop=(jt == NPJ - 1),
                        )
                    ot = opool.tile([P, 512], F32, tag="ot", name=f"ot{mt}_{tcn}")
                    nc.vector.tensor_scalar_add(
                        out=ot[:], in0=psp[:], scalar1=bproj_t[:, mt:mt + 1])
                    dma.dma_start(
                        out=out_ext[mt * P:(mt + 1) * P,
                                    tcn * 512:(tcn + 1) * 512],
                        in_=ot[:])

        wv_cm.__exit__(None, None, None)
        xpool_cm.__exit__(None, None, None)


def _emit_av(nc, vts, pso, p_, pt, o, kt, nkt):
    """attn@V for one (pair, kt) block: [ones|V_h].T @ P~ accumulated."""
    for h in range(2):
        head = 2 * p_ + h
        vaug = vts[kt][:, 128 * head:128 * head + 128]
        nc.tensor.matmul(
            pso[h][:, o:], vaug, pt[:, 512 * h + o:512 * (h + 1)],
            start=(kt == 0), stop=(kt == nkt - 1),
        )


def shard_inputs(x, W_qkv, b_qkv, W_proj, b_proj):
    """Build the 8 per-core input maps (host-side sharding)."""
    x = np.asarray(x, np.float32)
    W_qkv = np.asarray(W_qkv, np.float32)
    b_qkv = np.asarray(b_qkv, np.float32)
    W_proj = np.asarray(W_proj, np.float32)
    b_proj = np.asarray(b_proj, np.float32)
    in_maps = []
    for c in range(NCORES):
        b, g = c // 2, c % 2
        s = slice(512 * g, 512 * g + 512)
        Wq = W_qkv[0 * C:1 * C][s] * 0.125
        Wk = W_qkv[1 * C:2 * C][s]
        Wv = W_qkv[2 * C:3 * C][s]
        wqkv = np.ascontiguousarray(np.concatenate([Wq, Wk, Wv], 0).T)
        bq = b_qkv[0 * C:1 * C][s] * 0.125
        bk = b_qkv[1 * C:2 * C][s]
        bv = b_qkv[2 * C:3 * C][s]
        in_maps.append({
            "xT": np.ascontiguousarray(x[b].T),
            "wqkv": wqkv,
            "bqkv": np.ascontiguousarray(np.concatenate([bq, bk, bv])),
            "wproj": np.ascontiguousarray(W_proj[:, s].T).astype(ml_dtypes.bfloat16),
            "bproj": b_proj if g == 0 else np.zeros_like(b_proj),
        })
    return in_maps


def run(in_maps, trace=False):
    global _compiled
    if _compiled is None:
        _compiled = build()
    return run_bass_kernel_spmd(
        _compiled, in_maps, core_ids=list(range(NCORES)), trace=trace)


def kernel(x, W_qkv, b_qkv, W_proj, b_proj):
    in_maps = shard_inputs(x, W_qkv, b_qkv, W_proj, b_proj)
    res = run(in_maps)
    out = np.empty((B, T, C), np.float32)
    for b in range(B):
        partial = res.results[2 * b]["out"] + res.results[2 * b + 1]["out"]
        out[b] = partial.T
    return out


if __name__ == "__main__":
    rng = np.random.default_rng(0)
    xs = {
        "x": rng.standard_normal((B, T, C)).astype(np.float32),
        "W_qkv": (rng.standard_normal((3 * C, C)) / 32).astype(np.float32),
        "b_qkv": (rng.standard_normal(3 * C) * 0.02).astype(np.float32),
        "W_proj": (rng.standard_normal((C, C)) / 32).astype(np.float32),
        "b_proj": (rng.standard_normal(C) * 0.02).astype(np.float32),
    }
    out = kernel(**xs)
    print("out", out.shape, out.dtype, np.abs(out).mean())



# revision 3
# speedup vs baseline: 1.1572x; 1.1572x over previous
"""Distributed causal self-attention kernel for 8 Trainium2 NeuronCores.

Problem: B=4, T=2048, C=1024, H=16 heads, D=64 head dim, fp32.
  qkv = x @ W_qkv.T + b_qkv; causal attention per head; out = attn @ W_proj.T + b_proj

Sharding (hybrid DP x TP, no on-device collectives):
  core c -> batch b = c//2 (data parallel), head group g = c%2 (8 heads each,
  tensor parallel). Each core computes a row-parallel *partial* projection
  output for its batch; the host sums the two partials per batch (the TP
  reduction) and adds b_proj.

Engine plan (per core), all matmul operands bf16 (fp8 on the q/k/scores path
was measured at ~15% end-to-end error -- softmax output is a near-average, so
scores noise survives at full relative strength; bf16 keeps it at ~0.4%):
  - Q^T/K^T produced in [j, T] bf16 (weight-stationary GEMM, 1/8 folded into
    Wq); scores_T = K^T.T @ Q^T with the two heads of a pair on disjoint
    64-row PE groups -> they execute CONCURRENTLY (trace-verified row_grp
    pairs), so a pair's scores cost one head's cycles.
  - V in natural [T, j] bf16, stored per k-tile as [ones(64)|V_0..V_7]; the
    attn@V stationary [ones|V_h] emits the softmax denominator and the
    unnormalized output in one pass.
  - exp on the Scalar engine is the hard floor (~135-150us/core). The
    attention is swept qc-outer/pair-inner, and every independent PE unit
    (qk j-tile rounds, v-tiles, proj units) is WOVEN between the score and
    attn@V matmuls via a budgeted filler stream, so the in-order PE queue
    always has ACT-independent work while exp catches up. Projection for
    t-chunk i runs inside sweep i+1 instead of all at the tail.
  - diagonal-block causal masks run on GpSimd (Pool), off the DVE.
  - output partials DMA out as bf16; the host sums partials in f32.
"""
import sys

if "/opt/trn_rl_repo" not in sys.path:
    sys.path.insert(0, "/opt/trn_rl_repo")

import ml_dtypes
import numpy as np

import concourse.bass as bass
import concourse.tile as tile
from concourse import bacc, mybir
from concourse.bass_utils import run_bass_kernel_spmd
from concourse.masks import make_upper_triangular

F32 = mybir.dt.float32
BF16 = mybir.dt.bfloat16

B, T, C = 4, 2048, 1024
H, D = 16, 64
HC = 8            # heads per core
P = 128
NCORES = 8
NT = T // P       # 16 k-tiles
NTC = T // 512    # 4 t-chunks / q-chunks

_compiled = None


def build():
    nc = bacc.Bacc("TRN2", target_bir_lowering=False, debug=False,
                   num_devices=NCORES)
    xbf_ext = nc.declare_dram_parameter("xbf", [C, T], BF16, isOutput=False)
    wqk_ext = nc.declare_dram_parameter("wqk", [C, 1024], BF16, isOutput=False)
    bqk_ext = nc.declare_dram_parameter("bqk", [1024], F32, isOutput=False)
    wv_ext = nc.declare_dram_parameter("wv", [C, 512], BF16, isOutput=False)
    bv_ext = nc.declare_dram_parameter("bv", [512], F32, isOutput=False)
    wp_ext = nc.declare_dram_parameter("wp", [512, C], BF16, isOutput=False)
    bp_ext = nc.declare_dram_parameter("bp", [C], F32, isOutput=False)
    out_ext = nc.declare_dram_parameter("out", [C, T], BF16, isOutput=True)

    with tile.TileContext(nc, pool_alloc_mode="queue") as tc:
        _body(nc, tc, xbf_ext, wqk_ext, bqk_ext, wv_ext, bv_ext,
              wp_ext, bp_ext, out_ext)
    nc.compile()
    return nc


def _body(nc, tc, xbf_ext, wqk_ext, bqk_ext, wv_ext, bv_ext,
          wp_ext, bp_ext, out_ext):
    dma = nc.default_dma_engine
    Exp = mybir.ActivationFunctionType.Exp

    from contextlib import ExitStack
    ctx = ExitStack()
    with ctx:
        singles = ctx.enter_context(tc.tile_pool(name="singles", bufs=1))
        qkt_pool = ctx.enter_context(tc.tile_pool(name="qkT", bufs=1))
        vpool = ctx.enter_context(tc.tile_pool(name="v", bufs=1))
        apool = ctx.enter_context(tc.tile_pool(name="attnT", bufs=1))
        ptpool = ctx.enter_context(tc.tile_pool(name="pt", bufs=4))
        rspool = ctx.enter_context(tc.tile_pool(name="rs", bufs=4))
        wp_pool = ctx.enter_context(tc.tile_pool(name="wp", bufs=1))
        opool = ctx.enter_context(tc.tile_pool(name="outs", bufs=2))
        psum = ctx.enter_context(tc.tile_pool(name="psum", bufs=1, space="PSUM"))
        xpool = ctx.enter_context(tc.tile_pool(name="x", bufs=1, side="right"))
        wqk_pool = ctx.enter_context(tc.tile_pool(name="wqk", bufs=1, side="right"))

        # ---- HAM warmup: dummy fp32 matmuls bridge the DMA ramp so the PE
        # clock is at 8/8 when the first real matmuls issue.
        warm = rspool.tile([P, 512], F32, tag="rs", name="warm")
        nc.vector.memset(warm[:], 1.0)
        for i in range(6):
            wps = psum.tile([P, 512], F32, tag="mm", bufs=2, name=f"warm{i}")
            nc.tensor.matmul(wps[:], warm[:, 0:P], warm[:])

        # ---- constants ----
        mask = singles.tile([P, P], BF16)       # m[tk,tq]=1 iff tq >= tk
        make_upper_triangular(nc, mask[:], val=1.0, diag=True)
        mask_b = bass.AP(tensor=mask[:].tensor, offset=mask[:].offset,
                         ap=[mask[:].ap[0], [0, 2], [1, P]])

        bqk_t = singles.tile([P, 8], F32)       # per-partition q/k biases
        dma.dma_start(out=bqk_t[:], in_=bqk_ext[:].rearrange("(j p) -> p j", p=P))
        bv_b = singles.tile([P, 512], F32)      # v bias broadcast over partitions
        bv_src = bass.AP(tensor=bv_ext, offset=0, ap=[[0, P], [1, 512]])
        dma.dma_start(out=bv_b[:], in_=bv_src)
        bv_b3 = bv_b[:].rearrange("p (h c) -> p h c", h=HC)
        bproj_t = singles.tile([P, 8], F32)
        dma.dma_start(out=bproj_t[:], in_=bp_ext[:].rearrange("(m p) -> p m", p=P))

        # ---- weight + x loads (t-chunked so compute starts early) ----
        wqk = []
        for ct in range(8):
            wt = wqk_pool.tile([P, 1024], BF16, tag=f"wqk{ct}", name=f"wqk{ct}")
            dma.dma_start(out=wt[:], in_=wqk_ext[ct * P:(ct + 1) * P, :])
            wqk.append(wt)
        xts = [xpool.tile([P, T], BF16, tag=f"x{ct}", name=f"x{ct}")
               for ct in range(8)]
        for tcn in range(NTC):
            for ct in range(8):
                dma.dma_start(
                    out=xts[ct][:, tcn * 512:(tcn + 1) * 512],
                    in_=xbf_ext[ct * P:(ct + 1) * P, tcn * 512:(tcn + 1) * 512])
        wvt = []
        for ct in range(8):
            wt = wqk_pool.tile([P, 512], BF16, tag=f"wv{ct}", name=f"wv{ct}")
            dma.dma_start(out=wt[:], in_=wv_ext[ct * P:(ct + 1) * P, :])
            wvt.append(wt)
        wproj = []
        for jt in range(4):
            wt = wp_pool.tile([P, C], BF16, tag=f"wp{jt}", name=f"wp{jt}")
            dma.dma_start(out=wt[:], in_=wp_ext[jt * P:(jt + 1) * P, :])
            wproj.append(wt)

        # ---- persistent SBUF state ----
        # qkT[jt]: j-tile jt of [Q^T | K^T] in [j, T] bf16; q jt 0..3 (pairs),
        # k jt 4..7. Within a j-tile: partitions 0-63 head 2p, 64-127 head 2p+1.
        qkT = [qkt_pool.tile([P, T], BF16, tag=f"qk{jt}", name=f"qkT{jt}")
               for jt in range(8)]
        vts = [None] * NT
        attnT = [apool.tile([P, T], BF16, tag=f"a{p_}", name=f"attnT{p_}")
                 for p_ in range(4)]

        # ---- unit emitters ----
        def qk_unit(jt, tcn):
            sl = slice(tcn * 512, (tcn + 1) * 512)
            ps = psum.tile([P, 512], F32, tag="mm", bufs=2,
                           name=f"psqk{jt}_{tcn}")
            for ct in range(8):
                nc.tensor.matmul(ps[:], wqk[ct][:, jt * P:(jt + 1) * P],
                                 xts[ct][:, sl],
                                 start=(ct == 0), stop=(ct == 7))
            nc.vector.tensor_scalar_add(out=qkT[jt][:, sl], in0=ps[:],
                                        scalar1=bqk_t[:, jt:jt + 1])

        def v_tile(kt):
            psv = psum.tile([P, 512], F32, tag="mm", bufs=2, name=f"psv{kt}")
            for ct in range(8):
                nc.tensor.matmul(psv[:], xts[ct][:, kt * P:(kt + 1) * P],
                                 wvt[ct][:], start=(ct == 0), stop=(ct == 7))
            vt = vpool.tile([P, 1024], BF16, tag=f"v{kt}", name=f"v{kt}")
            vt3 = vt[:].rearrange("p (h c) -> p h c", h=HC)
            nc.vector.memset(vt3[:, :, 0:64], 1.0)
            nc.vector.tensor_add(vt3[:, :, 64:128],
                                 psv[:].rearrange("p (h c) -> p h c", h=HC),
                                 bv_b3)
            vts[kt] = vt

        def proj_unit(tcn, mt):
            sl = slice(tcn * 512, (tcn + 1) * 512)
            psp = psum.tile([P, 512], F32, tag="mm", bufs=2,
                            name=f"psp{mt}_{tcn}")
            for jt in range(4):
                nc.tensor.matmul(psp[:], wproj[jt][:, mt * P:(mt + 1) * P],
                                 attnT[jt][:, sl],
                                 start=(jt == 0), stop=(jt == 3))
            ot = opool.tile([P, 512], BF16, tag="ot", name=f"ot{mt}_{tcn}")
            nc.vector.tensor_scalar_add(out=ot[:], in0=psp[:],
                                        scalar1=bproj_t[:, mt:mt + 1])
            dma.dma_start(out=out_ext[mt * P:(mt + 1) * P, sl], in_=ot[:])

        # ---- filler stream: independent PE units woven into the attention
        # sweeps. (cost ~PE cycles, emit fn, min_sweep = earliest qc sweep)
        fillers = []
        for s in range(5):
            if s >= 1:
                for mt in range(8):
                    fillers.append(
                        (2300, (lambda t, m: lambda: proj_unit(t, m))(s - 1, mt), s))
            if s <= 2:
                tcn = s + 1
                for jt in (0, 4, 1, 5, 2, 6, 3, 7):
                    fillers.append(
                        (4400, (lambda j, t: lambda: qk_unit(j, t))(jt, tcn), s))
                for kt in range(4 * (s + 1), 4 * (s + 2)):
                    fillers.append(
                        (4400, (lambda k: lambda: v_tile(k))(kt), s))
        state = {"cur": 0, "budget": 0.0}

        def pull(sweep, add):
            state["budget"] = min(state["budget"] + add, 9000.0)
            while state["cur"] < len(fillers):
                cost, fn, ms = fillers[state["cur"]]
                if ms > sweep or state["budget"] < cost:
                    break
                fn()
                state["budget"] -= cost
                state["cur"] += 1

        def drain(through_sweep):
            while state["cur"] < len(fillers):
                cost, fn, ms = fillers[state["cur"]]
                if ms > through_sweep:
                    break
                fn()
                state["cur"] += 1
            state["budget"] = 0.0

        def emit_av(pso, p_, pt, o, kt, nkt):
            for h in range(2):
                head = 2 * p_ + h
                nc.tensor.matmul(pso[h][:, o:],
                                 vts[kt][:, 128 * head:128 * head + 128],
                                 pt[:, 512 * h + o:512 * (h + 1)],
                                 start=(kt == 0), stop=(kt == nkt - 1))

        def attn(p_, qc):
            qTt = qkT[p_]
            kTt = qkT[4 + p_]
            nkt = 4 * (qc + 1)
            pso = [psum.tile([P, 512], F32, tag="o", bufs=2,
                             name=f"pso{p_}_{qc}_{h}") for h in range(2)]
            pending = None
            for kt in range(nkt):
                o = max(0, kt * P - qc * 512)
                ss = psum.tile([P, 1024], F32, tag="s", bufs=2,
                               name=f"pss{p_}_{qc}_{kt}")
                for h in range(2):
                    lo = 64 * h
                    # the two heads sit on disjoint 64-row PE groups and
                    # execute concurrently
                    nc.tensor.matmul(
                        ss[:, 512 * h + o:512 * (h + 1)],
                        kTt[lo:lo + 64, kt * P:(kt + 1) * P],
                        qTt[lo:lo + 64, qc * 512 + o:(qc + 1) * 512])
                pt = ptpool.tile([P, 1024], BF16, tag="pt",
                                 name=f"pt{p_}_{qc}_{kt}")
                ss3 = ss[:].rearrange("p (h w) -> p h w", h=2)
                pt3 = pt[:].rearrange("p (h w) -> p h w", h=2)
                nc.scalar.activation(pt3[:, :, o:], ss3[:, :, o:], Exp)
                if kt >= 4 * qc:
                    nc.gpsimd.tensor_mul(pt3[:, :, o:o + P],
                                         pt3[:, :, o:o + P], mask_b)
                if pending is not None:
                    emit_av(pso, p_, *pending, nkt)
                pending = (pt, o, kt)
                pull(qc, 1.1 * (512 - o))
            emit_av(pso, p_, *pending, nkt)
            # normalize: pso rows 0:64 = denominator copies, 64:128 = outT
            for h in range(2):
                rsb = rspool.tile([P, 512], F32, tag="rs",
                                  name=f"rs{p_}_{qc}_{h}")
                nc.vector.reciprocal_approx_fast(rsb[0:64, :], pso[h][0:64, :])
                dma.dma_start(out=rsb[64:128, :], in_=rsb[0:64, :])
                lo = 64 * h
                nc.vector.tensor_mul(
                    attnT[p_][lo:lo + 64, qc * 512:(qc + 1) * 512],
                    pso[h][64:128, :], rsb[64:128, :])

        # ---- schedule: qk t-chunk 0, v 0..3, then qc-outer sweeps with
        # fillers (qk t-chunk i+1, v-tiles 4(i+1).., proj t-chunk i-1) ----
        for jt in (0, 4, 1, 5, 2, 6, 3, 7):
            qk_unit(jt, 0)
        for kt in range(4):
            v_tile(kt)
        for qc in range(NTC):
            drain(qc - 1)
            for p_ in range(4):
                attn(p_, qc)
        drain(4)


def shard_inputs(x, W_qkv, b_qkv, W_proj, b_proj):
    """Build the 8 per-core input maps (host-side sharding + layouts)."""
    x = np.asarray(x, np.float32)
    W_qkv = np.asarray(W_qkv, np.float32)
    b_qkv = np.asarray(b_qkv, np.float32)
    W_proj = np.asarray(W_proj, np.float32)
    b_proj = np.asarray(b_proj, np.float32)
    BF16NP = ml_dtypes.bfloat16

    in_maps = []
    for c in range(NCORES):
        b, g = c // 2, c % 2
        s = slice(512 * g, 512 * g + 512)
        Wq = W_qkv[0 * C:1 * C][s] * 0.125
        Wk = W_qkv[1 * C:2 * C][s]
        wqk = np.ascontiguousarray(np.concatenate([Wq, Wk], 0).T)
        bq = b_qkv[0 * C:1 * C][s] * 0.125
        bk = b_qkv[1 * C:2 * C][s]
        xT = x[b].T
        in_maps.append({
            "xbf": np.ascontiguousarray(xT).astype(BF16NP),
            "wqk": wqk.astype(BF16NP),
            "bqk": np.ascontiguousarray(np.concatenate([bq, bk])),
            "wv": np.ascontiguousarray(W_qkv[2 * C:3 * C][s].T).astype(BF16NP),
            "bv": np.ascontiguousarray(b_qkv[2 * C:3 * C][s]),
            "wp": np.ascontiguousarray(W_proj[:, s].T).astype(BF16NP),
            "bp": b_proj if g == 0 else np.zeros_like(b_proj),
        })
    return in_maps


def run(in_maps, trace=False):
    global _compiled
    if _compiled is None:
        _compiled = build()
    return run_bass_kernel_spmd(
        _compiled, in_maps, core_ids=list(range(NCORES)), trace=trace)


def kernel(x, W_qkv, b_qkv, W_proj, b_proj):
    in_maps = shard_inputs(x, W_qkv, b_qkv, W_proj, b_proj)
    res = run(in_maps)
    out = np.empty((B, T, C), np.float32)
    for b in range(B):
        partial = (res.results[2 * b]["out"].astype(np.float32)
                   + res.results[2 * b + 1]["out"].astype(np.float32))
        out[b] = partial.T
    return out


if __name__ == "__main__":
    rng = np.random.default_rng(0)
    xs = {
        "x": rng.standard_normal((B, T, C)).astype(np.float32),
        "W_qkv": (rng.standard_normal((3 * C, C)) / 32).astype(np.float32),
        "b_qkv": (rng.standard_normal(3 * C) * 0.02).astype(np.float32),
        "W_proj": (rng.standard_normal((C, C)) / 32).astype(np.float32),
        "b_proj": (rng.standard_normal(C) * 0.02).astype(np.float32),
    }
    out = kernel(**xs)
    print("out", out.shape, out.dtype, np.abs(out).mean())


# revision 10
# speedup vs baseline: 1.1989x; 1.0360x over previous
"""Distributed causal self-attention kernel for 8 Trainium2 NeuronCores.

Problem: B=4, T=2048, C=1024, H=16 heads, D=64 head dim, fp32.
  qkv = x @ W_qkv.T + b_qkv; causal attention per head; out = attn @ W_proj.T + b_proj

Sharding (hybrid DP x TP, no on-device collectives):
  core c -> batch b = c//2 (data parallel), head group g = c%2 (8 heads each,
  tensor parallel). Each core computes a row-parallel *partial* projection
  output for its batch; the host sums the two partials per batch (the TP
  reduction) and adds b_proj.

Engine plan (per core), all matmul operands bf16 (fp8 on the q/k/scores path
was measured at ~15% end-to-end error -- softmax output is a near-average, so
scores noise survives at full relative strength; bf16 keeps it at ~0.4%):
  - Q^T/K^T produced in [j, T] bf16 (weight-stationary GEMM, 1/8 folded into
    Wq); scores_T = K^T.T @ Q^T with the two heads of a pair on disjoint
    64-row PE groups -> they execute CONCURRENTLY (trace-verified row_grp
    pairs), so a pair's scores cost one head's cycles.
  - V in natural [T, j] bf16, stored per k-tile as [ones(64)|V_0..V_7]; the
    attn@V stationary [ones|V_h] emits the softmax denominator and the
    unnormalized output in one pass.
  - exp on the Scalar engine is the hard floor (~135-150us/core). The
    attention is swept qc-outer/pair-inner, and every independent PE unit
    (qk j-tile rounds, v-tiles, proj units) is WOVEN between the score and
    attn@V matmuls via a budgeted filler stream, so the in-order PE queue
    always has ACT-independent work while exp catches up. Projection for
    t-chunk i runs inside sweep i+1 instead of all at the tail.
  - diagonal-block causal masks run on GpSimd (Pool), off the DVE.
  - output partials DMA out as bf16; the host sums partials in f32.
"""
import sys

if "/opt/trn_rl_repo" not in sys.path:
    sys.path.insert(0, "/opt/trn_rl_repo")

import ml_dtypes
import numpy as np

import concourse.bass as bass
import concourse.tile as tile
from concourse import bacc, mybir
from concourse.bass_utils import run_bass_kernel_spmd
from concourse.masks import make_upper_triangular

F32 = mybir.dt.float32
BF16 = mybir.dt.bfloat16

B, T, C = 4, 2048, 1024
H, D = 16, 64
HC = 8            # heads per core
P = 128
NCORES = 8
NT = T // P       # 16 k-tiles
NTC = T // 512    # 4 t-chunks / q-chunks

_compiled = None


def build():
    nc = bacc.Bacc("TRN2", target_bir_lowering=False, debug=False,
                   num_devices=NCORES)
    xbf_ext = nc.declare_dram_parameter("xbf", [C, T], BF16, isOutput=False)
    wqk_ext = nc.declare_dram_parameter("wqk", [C, 1024], BF16, isOutput=False)
    bqk_ext = nc.declare_dram_parameter("bqk", [1024], F32, isOutput=False)
    wv_ext = nc.declare_dram_parameter("wv", [C, 512], BF16, isOutput=False)
    bv_ext = nc.declare_dram_parameter("bv", [512], F32, isOutput=False)
    wp_ext = nc.declare_dram_parameter("wp", [512, C], BF16, isOutput=False)
    bp_ext = nc.declare_dram_parameter("bp", [C], F32, isOutput=False)
    out_ext = nc.declare_dram_parameter("out", [C, T], BF16, isOutput=True)

    with tile.TileContext(nc, pool_alloc_mode="queue") as tc:
        _body(nc, tc, xbf_ext, wqk_ext, bqk_ext, wv_ext, bv_ext,
              wp_ext, bp_ext, out_ext)
    nc.compile()
    return nc


def _body(nc, tc, xbf_ext, wqk_ext, bqk_ext, wv_ext, bv_ext,
          wp_ext, bp_ext, out_ext):
    dma = nc.default_dma_engine
    Exp = mybir.ActivationFunctionType.Exp

    from contextlib import ExitStack
    ctx = ExitStack()
    with ctx:
        singles = ctx.enter_context(tc.tile_pool(name="singles", bufs=1))
        qkt_pool = ctx.enter_context(tc.tile_pool(name="qkT", bufs=1))
        vpool = ctx.enter_context(tc.tile_pool(name="v", bufs=1))
        apool = ctx.enter_context(tc.tile_pool(name="attnT", bufs=1))
        ptpool = ctx.enter_context(tc.tile_pool(name="pt", bufs=4))
        rspool = ctx.enter_context(tc.tile_pool(name="rs", bufs=4))
        wp_pool = ctx.enter_context(tc.tile_pool(name="wp", bufs=1))
        opool = ctx.enter_context(tc.tile_pool(name="outs", bufs=2))
        psum = ctx.enter_context(tc.tile_pool(name="psum", bufs=1, space="PSUM"))
        xpool = ctx.enter_context(tc.tile_pool(name="x", bufs=1, side="right"))
        wqk_pool = ctx.enter_context(tc.tile_pool(name="wqk", bufs=1, side="right"))

        # ---- HAM warmup: dummy fp32 matmuls bridge the DMA ramp so the PE
        # clock is at 8/8 when the first real matmuls issue.
        warm = rspool.tile([P, 512], F32, tag="rs", name="warm")
        nc.vector.memset(warm[:], 1.0)
        for i in range(6):
            wps = psum.tile([P, 512], F32, tag="mm", bufs=2, name=f"warm{i}")
            nc.tensor.matmul(wps[:], warm[:, 0:P], warm[:])

        # ---- constants ----
        mask = singles.tile([P, P], BF16)       # m[tk,tq]=1 iff tq >= tk
        make_upper_triangular(nc, mask[:], val=1.0, diag=True)
        mask_b = bass.AP(tensor=mask[:].tensor, offset=mask[:].offset,
                         ap=[mask[:].ap[0], [0, 2], [1, P]])

        bqk_t = singles.tile([P, 8], F32)       # per-partition q/k biases
        dma.dma_start(out=bqk_t[:], in_=bqk_ext[:].rearrange("(j p) -> p j", p=P))
        bv_b = singles.tile([P, 512], F32)      # v bias broadcast over partitions
        bv_src = bass.AP(tensor=bv_ext, offset=0, ap=[[0, P], [1, 512]])
        dma.dma_start(out=bv_b[:], in_=bv_src)
        bv_b3 = bv_b[:].rearrange("p (h c) -> p h c", h=HC)
        bproj_t = singles.tile([P, 8], F32)
        dma.dma_start(out=bproj_t[:], in_=bp_ext[:].rearrange("(m p) -> p m", p=P))

        # ---- weight + x loads (t-chunked so compute starts early) ----
        wqk = []
        for ct in range(8):
            wt = wqk_pool.tile([P, 1024], BF16, tag=f"wqk{ct}", name=f"wqk{ct}")
            dma.dma_start(out=wt[:], in_=wqk_ext[ct * P:(ct + 1) * P, :])
            wqk.append(wt)
        xts = [xpool.tile([P, T], BF16, tag=f"x{ct}", name=f"x{ct}")
               for ct in range(8)]
        for ct in range(8):
            dma.dma_start(out=xts[ct][:, 0:512],
                          in_=xbf_ext[ct * P:(ct + 1) * P, 0:512])
        wvt = []
        for ct in range(8):
            wt = wqk_pool.tile([P, 512], BF16, tag=f"wv{ct}", name=f"wv{ct}")
            dma.dma_start(out=wt[:], in_=wv_ext[ct * P:(ct + 1) * P, :])
            wvt.append(wt)
        for tcn in range(1, NTC):
            for ct in range(8):
                dma.dma_start(
                    out=xts[ct][:, tcn * 512:(tcn + 1) * 512],
                    in_=xbf_ext[ct * P:(ct + 1) * P, tcn * 512:(tcn + 1) * 512])
        wproj = []
        for jt in range(4):
            wt = wp_pool.tile([P, C], BF16, tag=f"wp{jt}", name=f"wp{jt}")
            dma.dma_start(out=wt[:], in_=wp_ext[jt * P:(jt + 1) * P, :])
            wproj.append(wt)

        # ---- persistent SBUF state ----
        # qkT[jt]: j-tile jt of [Q^T | K^T] in [j, T] bf16; q jt 0..3 (pairs),
        # k jt 4..7. Within a j-tile: partitions 0-63 head 2p, 64-127 head 2p+1.
        qkT = [qkt_pool.tile([P, T], BF16, tag=f"qk{jt}", name=f"qkT{jt}")
               for jt in range(8)]
        vts = [None] * NT
        attnT = [apool.tile([P, T], BF16, tag=f"a{p_}", name=f"attnT{p_}")
                 for p_ in range(4)]

        # ---- unit emitters ----
        def qk_unit(jt, tcn):
            sl = slice(tcn * 512, (tcn + 1) * 512)
            ps = psum.tile([P, 512], F32, tag="mm", bufs=2,
                           name=f"psqk{jt}_{tcn}")
            for ct in range(8):
                nc.tensor.matmul(ps[:], wqk[ct][:, jt * P:(jt + 1) * P],
                                 xts[ct][:, sl],
                                 start=(ct == 0), stop=(ct == 7))
            nc.vector.tensor_scalar_add(out=qkT[jt][:, sl], in0=ps[:],
                                        scalar1=bqk_t[:, jt:jt + 1])

        def v_tile(kt):
            psv = psum.tile([P, 512], F32, tag="mm", bufs=2, name=f"psv{kt}")
            for ct in range(8):
                nc.tensor.matmul(psv[:], xts[ct][:, kt * P:(kt + 1) * P],
                                 wvt[ct][:], start=(ct == 0), stop=(ct == 7))
            vt = vpool.tile([P, 1024], BF16, tag=f"v{kt}", name=f"v{kt}")
            vt3 = vt[:].rearrange("p (h c) -> p h c", h=HC)
            nc.vector.memset(vt3[:, :, 0:64], 1.0)
            nc.vector.tensor_add(vt3[:, :, 64:128],
                                 psv[:].rearrange("p (h c) -> p h c", h=HC),
                                 bv_b3)
            vts[kt] = vt

        def proj_unit(tcn, mt):
            sl = slice(tcn * 512, (tcn + 1) * 512)
            psp = psum.tile([P, 512], F32, tag="mm", bufs=2,
                            name=f"psp{mt}_{tcn}")
            for jt in range(4):
                nc.tensor.matmul(psp[:], wproj[jt][:, mt * P:(mt + 1) * P],
                                 attnT[jt][:, sl],
                                 start=(jt == 0), stop=(jt == 3))
            ot = opool.tile([P, 512], BF16, tag="ot", name=f"ot{mt}_{tcn}")
            nc.vector.tensor_scalar_add(out=ot[:], in0=psp[:],
                                        scalar1=bproj_t[:, mt:mt + 1])
            dma.dma_start(out=out_ext[mt * P:(mt + 1) * P, sl], in_=ot[:])

        # ---- filler stream: independent PE units woven into the attention
        # sweeps, deadline-scheduled so each unit lands just before its first
        # consumer instead of in a burst at sweep boundaries (which starves
        # the exp pipeline behind the in-order PE queue).
        TPT = 17          # time slots per (sweep, pair): kt 0..16
        END = 4 * 4 * TPT + 100   # strictly beyond any pull's now+lookahead

        def tpt(qc, p_, kt):
            return (qc * 4 + p_) * TPT + kt

        fillers = []      # (deadline, ready, cost, fn)
        for s in range(NTC):
            for p_ in range(4):
                fillers.append((tpt(s, p_, 0), 0, 4400,
                                (lambda j, t: lambda: qk_unit(j, t))(p_, s)))
                fillers.append((tpt(s, p_, max(0, 4 * s - 1)), 0, 4400,
                                (lambda j, t: lambda: qk_unit(j, t))(4 + p_, s)))
            for kt in range(4 * s, 4 * s + 4):
                fillers.append((tpt(s, 0, kt), 0, 4400,
                                (lambda k: lambda: v_tile(k))(kt)))
        for tcn in range(NTC):
            for mt in range(8):
                # woven near the end of pair mt//2's kt loop in sweep tcn+1
                # (forced there via the deadline); tcn=3 drains at the tail
                dl = tpt(tcn + 1, mt // 2, 4 * (tcn + 2)) if tcn < 3 else END
                fillers.append((dl, tpt(tcn + 1, 0, 0), 2300,
                                (lambda t, m: lambda: proj_unit(t, m))(tcn, mt)))
        fillers.sort(key=lambda u: (u[0], u[1]))
        state = {"budget": 0.0}

        def pull(now, add):
            state["budget"] = min(state["budget"] + add, 9000.0)
            i = 0
            while i < len(fillers):
                dl, ready, cost, fn = fillers[i]
                forced = dl <= now + 2
                if forced or (ready <= now and state["budget"] >= cost):
                    fn()
                    if not forced:
                        state["budget"] -= cost
                    fillers.pop(i)
                elif dl > now + 2 and ready > now:
                    i += 1
                else:
                    break

        def emit_av(pso, p_, pt, o, kt, nkt):
            for h in range(2):
                head = 2 * p_ + h
                nc.tensor.matmul(pso[h][:, o:],
                                 vts[kt][:, 128 * head:128 * head + 128],
                                 pt[:, 512 * h + o:512 * (h + 1)],
                                 start=(kt == 0), stop=(kt == nkt - 1))

        def attn(p_, qc):
            qTt = qkT[p_]
            kTt = qkT[4 + p_]
            nkt = 4 * (qc + 1)
            pso = [psum.tile([P, 512], F32, tag="o", bufs=2,
                             name=f"pso{p_}_{qc}_{h}") for h in range(2)]
            pending = None
            for kt in range(nkt):
                pull(tpt(qc, p_, kt), 1.1 * (512 - max(0, kt * P - qc * 512)))
                o = max(0, kt * P - qc * 512)
                ss = psum.tile([P, 1024], F32, tag="s", bufs=2,
                               name=f"pss{p_}_{qc}_{kt}")
                for h in range(2):
                    lo = 64 * h
                    # the two heads sit on disjoint 64-row PE groups and
                    # execute concurrently
                    nc.tensor.matmul(
                        ss[:, 512 * h + o:512 * (h + 1)],
                        kTt[lo:lo + 64, kt * P:(kt + 1) * P],
                        qTt[lo:lo + 64, qc * 512 + o:(qc + 1) * 512])
                pt = ptpool.tile([P, 1024], BF16, tag="pt",
                                 name=f"pt{p_}_{qc}_{kt}")
                ss3 = ss[:].rearrange("p (h w) -> p h w", h=2)
                pt3 = pt[:].rearrange("p (h w) -> p h w", h=2)
                nc.scalar.activation(pt3[:, :, o:], ss3[:, :, o:], Exp)
                if kt >= 4 * qc:
                    nc.gpsimd.tensor_mul(pt3[:, :, o:o + P],
                                         pt3[:, :, o:o + P], mask_b)
                if pending is not None:
                    emit_av(pso, p_, *pending, nkt)
                pending = (pt, o, kt)
            emit_av(pso, p_, *pending, nkt)
            # normalize: pso rows 0:64 = denominator copies, 64:128 = outT
            for h in range(2):
                rsb = rspool.tile([P, 512], F32, tag="rs",
                                  name=f"rs{p_}_{qc}_{h}")
                nc.vector.reciprocal_approx_fast(rsb[0:64, :], pso[h][0:64, :])
                dma.dma_start(out=rsb[64:128, :], in_=rsb[0:64, :])
                lo = 64 * h
                nc.vector.tensor_mul(
                    attnT[p_][lo:lo + 64, qc * 512:(qc + 1) * 512],
                    pso[h][64:128, :], rsb[64:128, :])

        # ---- schedule: qc-outer sweeps; qk rounds, v-tiles and proj units
        # all arrive just-in-time through the deadline-driven filler pulls ----
        for qc in range(NTC):
            for p_ in range(4):
                attn(p_, qc)
        while fillers:
            fillers.pop(0)[3]()


def shard_inputs(x, W_qkv, b_qkv, W_proj, b_proj):
    """Build the 8 per-core input maps (host-side sharding + layouts)."""
    x = np.asarray(x, np.float32)
    W_qkv = np.asarray(W_qkv, np.float32)
    b_qkv = np.asarray(b_qkv, np.float32)
    W_proj = np.asarray(W_proj, np.float32)
    b_proj = np.asarray(b_proj, np.float32)
    BF16NP = ml_dtypes.bfloat16

    in_maps = []
    for c in range(NCORES):
        b, g = c // 2, c % 2
        s = slice(512 * g, 512 * g + 512)
        Wq = W_qkv[0 * C:1 * C][s] * 0.125
        Wk = W_qkv[1 * C:2 * C][s]
        wqk = np.ascontiguousarray(np.concatenate([Wq, Wk], 0).T)
        bq = b_qkv[0 * C:1 * C][s] * 0.125
        bk = b_qkv[1 * C:2 * C][s]
        xT = x[b].T
        in_maps.append({
            "xbf": np.ascontiguousarray(xT).astype(BF16NP),
            "wqk": wqk.astype(BF16NP),
            "bqk": np.ascontiguousarray(np.concatenate([bq, bk])),
            "wv": np.ascontiguousarray(W_qkv[2 * C:3 * C][s].T).astype(BF16NP),
            "bv": np.ascontiguousarray(b_qkv[2 * C:3 * C][s]),
            "wp": np.ascontiguousarray(W_proj[:, s].T).astype(BF16NP),
            "bp": b_proj if g == 0 else np.zeros_like(b_proj),
        })
    return in_maps


def run(in_maps, trace=False):
    global _compiled
    if _compiled is None:
        _compiled = build()
    return run_bass_kernel_spmd(
        _compiled, in_maps, core_ids=list(range(NCORES)), trace=trace)


def kernel(x, W_qkv, b_qkv, W_proj, b_proj):
    in_maps = shard_inputs(x, W_qkv, b_qkv, W_proj, b_proj)
    res = run(in_maps)
    out = np.empty((B, T, C), np.float32)
    for b in range(B):
        partial = (res.results[2 * b]["out"].astype(np.float32)
                   + res.results[2 * b + 1]["out"].astype(np.float32))
        out[b] = partial.T
    return out


if __name__ == "__main__":
    rng = np.random.default_rng(0)
    xs = {
        "x": rng.standard_normal((B, T, C)).astype(np.float32),
        "W_qkv": (rng.standard_normal((3 * C, C)) / 32).astype(np.float32),
        "b_qkv": (rng.standard_normal(3 * C) * 0.02).astype(np.float32),
        "W_proj": (rng.standard_normal((C, C)) / 32).astype(np.float32),
        "b_proj": (rng.standard_normal(C) * 0.02).astype(np.float32),
    }
    out = kernel(**xs)
    print("out", out.shape, out.dtype, np.abs(out).mean())
